# revision 45
# baseline (speedup 1.0000x reference)
"""TRN2 Bass kernel for GPT-2 style causal self-attention (B=4, S=2048, D=1024, H=16).

Sharding: 8 cores = 4 batches x 2 head-groups (8 heads each).
Each core computes qkv projections for its (batch, head-group), runs causal
attention for its 8 heads, then the two head-groups' attention outputs are
exchanged pairwise (AllGather per head-pair, spread through the kernel) so each
core computes the c_proj over the FULL 1024 a-channels for its half of the
OUTPUT COLUMNS (wp is pre-sliced per core host-side) -- no collective remains
in the kernel tail.

All matmul operands are bf16 (fp32 PSUM accumulation): same PE rate as f32r but
fast-weight-load kicks in and DMA halves. Weights are preloaded to SBUF once.
Softmax needs no max-subtraction (scores bounded ~|2.7|); masked entries are
zeroed after exp via affine_select restricted to the 128-col diagonal band; the
softmax denominator rides along as a 65th ones-column of V in the AV matmul.
AV matmuls skip fully-masked query columns of diagonal key tiles.
Attention is software-pipelined (scores 2 tiles ahead of AV) and score matmuls
for a head pair run concurrently on disjoint PE row groups via tile_position.
"""
import sys
sys.path.insert(0, "/opt/trn_rl_repo")
import numpy as np

B, S, D, H, HD = 4, 2048, 1024, 16, 64
NCORES = 8
HPC = H // 2          # 8 heads per core
ACH = HPC * HD        # 512 local a-channels
P = 128
QCN = 4               # token chunks
QCS = S // QCN        # 512
FKT = D // P          # 8 feature k-tiles
VW = HPC * (HD + 1)   # 520: per-head 64 v-dims + ones column
SKEW = 3              # attention pipeline skew (score tiles ahead of AV)
OCW = D // 2          # 512 output columns per core

_CACHE = {}


def _build():
    from concourse import bacc, tile, mybir
    f32 = mybir.dt.float32
    bf16 = mybir.dt.bfloat16
    Exp = mybir.ActivationFunctionType.Exp

    nc = bacc.Bacc("TRN2", target_bir_lowering=False, debug=False,
                   num_devices=NCORES)
    xt_e = nc.dram_tensor("xt", [D, S], bf16, kind="ExternalInput")
    wq_e = nc.dram_tensor("wq", [D, ACH], bf16, kind="ExternalInput")
    wk_e = nc.dram_tensor("wk", [D, ACH], bf16, kind="ExternalInput")
    wv_e = nc.dram_tensor("wv", [D, ACH], bf16, kind="ExternalInput")
    wp_e = nc.dram_tensor("wp", [D, OCW], bf16, kind="ExternalInput")
    out_e = nc.dram_tensor("outp", [S, OCW], bf16, kind="ExternalOutput")
    rg = [[0, 1], [2, 3], [4, 5], [6, 7]]

    with tile.TileContext(nc) as tc:
        with tc.tile_pool(name="sb", bufs=1) as sb, \
             tc.tile_pool(name="pp", bufs=1, space="PSUM") as pp, \
             tc.tile_pool(name="dr", bufs=1, space="DRAM") as dr:

            kT = [sb.tile([P, S], bf16, name=f"kTr{i}", tag="kT", bufs=4)
                  for i in range(4)]
            vx = [sb.tile([P, VW], bf16, name=f"vxr{i}", tag="vx", bufs=16)
                  for i in range(16)]
            wq_t = [sb.tile([P, ACH], bf16, name=f"wqr{i}", tag="wq", bufs=8)
                    for i in range(FKT)]
            wk_t = [sb.tile([P, ACH], bf16, name=f"wkr{i}", tag="wk", bufs=8)
                    for i in range(FKT)]
            wv_t = [sb.tile([P, ACH], bf16, name=f"wvr{i}", tag="wv", bufs=8)
                    for i in range(FKT)]
            wp_t = [sb.tile([P, OCW], bf16, name=f"wpr{i}", tag="wp", bufs=8)
                    for i in range(FKT)]

            qt_all = {}    # (qc, ct) -> tile
            aods = {}      # (qc, hp) -> dram tile [2P, TPC]

            def preload_weights():
                for k in range(FKT):
                    nc.scalar.dma_start(out=wq_t[k],
                                        in_=wq_e.ap()[k * P:(k + 1) * P, :])
                for k in range(FKT):
                    nc.gpsimd.dma_start(out=wk_t[k],
                                        in_=wk_e.ap()[k * P:(k + 1) * P, :])
                for k in range(FKT):
                    nc.gpsimd.dma_start(out=wv_t[k],
                                        in_=wv_e.ap()[k * P:(k + 1) * P, :])
                for k in range(FKT):
                    nc.gpsimd.dma_start(out=wp_t[k],
                                        in_=wp_e.ap()[k * P:(k + 1) * P, :])

            _xc = {}

            def qk_group(qc, proj, ct):
                w_t = wq_t if proj == "q" else wk_t
                xc = _xc[qc]
                mm_ps = pp.tile([P, QCS], f32, name=f"{proj}ps{qc}_{ct}",
                                tag="mm1", bufs=2)
                for k in range(FKT):
                    def mm(k=k, mm_ps=mm_ps, w_t=w_t, ct=ct, xck=xc[k]):
                        nc.tensor.matmul(
                            mm_ps[:, :], w_t[k][:, ct * P:(ct + 1) * P],
                            xck[:, :], start=(k == 0), stop=(k == FKT - 1))
                    yield mm
                if proj == "q":
                    qt = sb.tile([P, QCS], bf16, name=f"qt{qc}_{ct}",
                                 tag="qt", bufs=8)
                    qt_all[qc, ct] = qt

                    def cp(qt=qt, mm_ps=mm_ps):
                        nc.vector.tensor_copy(out=qt, in_=mm_ps)
                    yield cp
                else:
                    def cp(ct=ct, mm_ps=mm_ps, qc=qc):
                        nc.vector.tensor_copy(
                            out=kT[ct][:, qc * QCS:(qc + 1) * QCS],
                            in_=mm_ps)
                    yield cp

            def v_group(qc, vt):
                xc = _xc[qc]
                v_ps = pp.tile([P, ACH], f32, name=f"vps{qc}_{vt}",
                               tag="mm1", bufs=2)
                for k in range(FKT):
                    def mm(k=k, v_ps=v_ps, xck=xc[k], vt=vt):
                        nc.tensor.matmul(v_ps[:, :],
                                         xck[:, vt * P:(vt + 1) * P],
                                         wv_t[k][:, :], start=(k == 0),
                                         stop=(k == FKT - 1))
                    yield mm

                def vcp(qc=qc, vt=vt, v_ps=v_ps):
                    vxt = vx[qc * 4 + vt]
                    v3 = vxt.rearrange("p (h w) -> p h w", w=HD + 1)
                    nc.gpsimd.memset(v3[:, :, HD:HD + 1], 1.0)
                    nc.vector.tensor_copy(
                        out=v3[:, :, 0:HD],
                        in_=v_ps.rearrange("p (h d) -> p h d", d=HD))
                yield vcp

            def prefetch_x():
                """All of x is only 4MB bf16: load every chunk up front so
                the sync queue carries nothing but cproj readbacks later."""
                for qc in range(QCN):
                    _xc[qc] = [sb.tile([P, QCS], bf16, name=f"xc{qc}_{k}",
                                       tag="xc", bufs=32)
                               for k in range(FKT)]
                for qc in range(QCN):
                    for k in range(FKT):
                        nc.sync.dma_start(
                            out=_xc[qc][k],
                            in_=xt_e.ap()[k * P:(k + 1) * P,
                                          qc * QCS:(qc + 1) * QCS])

            def qkv_units(qc, part="all"):
                """Emission closures for the qkv phase of qc. part='early'
                emits q projections + k ct0; part='late' the rest (v first,
                then k ct1-3) -- deferred into attention(qc) where they are
                not needed before key-tile 4qc."""
                if part == "all":
                    for ct in range(4):
                        yield from qk_group(qc, "q", ct)
                    for ct in range(4):
                        yield from qk_group(qc, "k", ct)
                    for vt in range(4):
                        yield from v_group(qc, vt)
                elif part == "early":
                    for ct in range(4):
                        yield from qk_group(qc, "q", ct)
                    yield from qk_group(qc, "k", 0)
                else:
                    for vt in range(4):
                        yield from v_group(qc, vt)
                    for ct in range(1, 4):
                        yield from qk_group(qc, "k", ct)

            def cproj_units(qc):
                """Generator of closures for c_proj of qc: read back the
                AllGather'd attention outputs (both head-groups, all 512
                tokens) and compute c_proj for this core's OCW out-columns."""
                ao = {}
                for j in range(2):
                    for hp in range(4):
                        ao[j, hp] = sb.tile([P, QCS], bf16,
                                            name=f"ao{qc}_{j}_{hp}",
                                            tag="ao", bufs=16)

                def rb(hp):
                    for j in range(2):
                        nc.sync.dma_start(
                            out=ao[j, hp],
                            in_=aods[qc, hp][j * P:(j + 1) * P, :])
                for hp in range(4):
                    yield lambda hp=hp: rb(hp)

                def st_(qc, tt, po):
                    pout = sb.tile([P, OCW], bf16, name=f"pout{qc}_{tt}",
                                   tag="pout", bufs=4)
                    nc.vector.tensor_copy(out=pout, in_=po)
                    nc.gpsimd.dma_start(
                        out=out_e.ap()[qc * QCS + tt * P:
                                       qc * QCS + (tt + 1) * P, :],
                        in_=pout)

                if qc < QCN - 1:
                    # tt-major: AllGathers finished long ago, 8-matmul
                    # bursts per PSUM tile keep the mm1 pool fluid
                    for tt in range(4):
                        po = pp.tile([P, OCW], f32, name=f"po{qc}_{tt}",
                                     tag="mm1", bufs=2)
                        for hp in range(4):
                            for j in range(2):
                                def mm(j=j, hp=hp, po=po, tt=tt):
                                    nc.tensor.matmul(
                                        po[:, :],
                                        ao[j, hp][:, tt * P:(tt + 1) * P],
                                        wp_t[j * 4 + hp][:, :],
                                        start=(hp == 0 and j == 0),
                                        stop=(hp == 3 and j == 1))
                                yield mm
                        yield lambda qc=qc, tt=tt, po=po: st_(qc, tt, po)
                else:
                    # tail chunk: hp-major over tt pairs so only the final
                    # head pair's matmuls wait on the last AllGather
                    for tp in range(2):
                        pos = {tt: pp.tile([P, OCW], f32,
                                           name=f"po{qc}_{tt}", tag="mm1",
                                           bufs=2)
                               for tt in (2 * tp, 2 * tp + 1)}
                        for hp in range(4):
                            if tp == 0 and hp == 3:
                                # warm-keeper: the next matmuls wait on the
                                # last AllGather; keep the HAM clock gate
                                # open with dummy matmuls meanwhile
                                def wk():
                                    for w in range(48):
                                        dps = pp.tile(
                                            [65, QCS], f32,
                                            name=f"dps{qc}_{w}",
                                            tag="acc", bufs=2)
                                        nc.tensor.matmul(
                                            dps[:, :], wrm[:, 0:65],
                                            wrm[:, :], start=True,
                                            stop=True)
                                yield wk
                            for j in range(2):
                                for tt in (2 * tp, 2 * tp + 1):
                                    def mm(j=j, hp=hp, tt=tt, pos=pos):
                                        nc.tensor.matmul(
                                            pos[tt][:, :],
                                            ao[j, hp][:, tt * P:
                                                      (tt + 1) * P],
                                            wp_t[j * 4 + hp][:, :],
                                            start=(hp == 0 and j == 0),
                                            stop=(hp == 3 and j == 1))
                                    yield mm
                        for tt in (2 * tp, 2 * tp + 1):
                            yield (lambda qc=qc, tt=tt, pos=pos:
                                   st_(qc, tt, pos[tt]))

            def emit_attention(qc, fillers, rate=2.2):
                """Emit attention for qc, interleaving filler closures at
                ~rate units per pipeline step. Each head pair's softmax
                normalization + partner-exchange is deferred into the next
                head pair's early steps: only a fast acc->SBUF copy happens
                at the block boundary, so the PSUM acc pool and the gpsimd
                queue never stall the next block's matmuls/selects."""
                nkt = 4 * qc + 4
                fi = 0
                budget = 0.0
                at_tiles = [sb.tile([P, QCS], bf16, name=f"at{qc}_{j}",
                                    tag="at", bufs=6) for j in range(4)]
                deferred = []

                def mk_norm(hp, h, half, accS_h):
                    def rb_():
                        rsum = sb.tile([1, QCS], f32, name=f"rsum{qc}_{h}",
                                       tag="rs", bufs=4)
                        rs_t = sb.tile([1, QCS], f32, name=f"rst{qc}_{h}",
                                       tag="rs2", bufs=4)
                        rb_t = sb.tile([64, QCS], f32, name=f"rb{qc}_{h}",
                                       tag="rb", bufs=4)
                        # recip is a custom DVE op: give it a partition-0
                        # based operand, not a base_partition=64 slice
                        nc.vector.tensor_copy(out=rsum,
                                              in_=accS_h[64:65, :])
                        nc.vector.reciprocal_approx_fast(
                            out=rs_t, in_=rsum)
                        nc.gpsimd.partition_broadcast(rb_t[:, :],
                                                      rs_t[:, :])
                        nc.vector.tensor_tensor(
                            out=at_tiles[hp][half:half + 64, :],
                            in0=accS_h[0:64, :], in1=rb_t[:, :],
                            op=mybir.AluOpType.mult)
                    return rb_

                def mk_ship(hp):
                    def ship():
                        # AllGather -> aod rows [0:P] = member 0's
                        # (head-group 0), rows [P:2P] = member 1's.
                        atd = dr.tile([P, QCS], bf16, name=f"atd{qc}_{hp}",
                                      tag=f"atd{qc}_{hp}")
                        aod = dr.tile([2 * P, QCS], bf16,
                                      name=f"aod{qc}_{hp}",
                                      tag=f"aod{qc}_{hp}")
                        nc.gpsimd.dma_start(out=atd[:, :], in_=at_tiles[hp])
                        nc.gpsimd.collective_compute(
                            "AllGather", mybir.AluOpType.bypass,
                            ins=[atd.opt()], outs=[aod.opt()],
                            replica_groups=rg)
                        aods[qc, hp] = aod
                    return ship

                for hp in range(4):
                    h_e, h_o = 2 * hp, 2 * hp + 1
                    acc = {}
                    for h, half in ((h_e, 0), (h_o, 64)):
                        acc[h] = pp.tile([65, QCS], f32, name=f"acc{qc}_{h}",
                                         tag="acc", bufs=2)
                    pts = {}
                    for step in range(nkt + SKEW):
                        if step < nkt:
                            kt = step
                            # both heads' score tiles share one 2-bank PSUM
                            # tile; a single exp covers the pair
                            st = pp.tile([P, 2 * QCS], f32,
                                         name=f"st{qc}_{hp}_{kt}",
                                         tag="st", bufs=2)
                            for h, half in ((h_e, 0), (h_o, 64)):
                                nc.tensor.matmul(
                                    st[:, half * 8:half * 8 + QCS],
                                    kT[hp][half:half + 64,
                                           kt * P:(kt + 1) * P],
                                    qt_all[qc, hp][half:half + 64, :],
                                    start=True, stop=True,
                                    tile_position=(half, 0))
                            pt = sb.tile([P, 2 * QCS], bf16,
                                         name=f"pt{qc}_{hp}_{kt}",
                                         tag="pt", bufs=5)
                            nc.scalar.activation(out=pt, in_=st,
                                                 func=Exp, scale=0.125)
                            if kt >= 4 * qc:
                                off = (kt - 4 * qc) * P
                                for half in (0, 64):
                                    nc.gpsimd.affine_select(
                                        out=pt[:, half * 8 + off:
                                               half * 8 + off + P],
                                        in_=pt[:, half * 8 + off:
                                               half * 8 + off + P],
                                        compare_op=mybir.AluOpType.is_ge,
                                        fill=0.0, base=0,
                                        pattern=[[1, P]],
                                        channel_multiplier=-1)
                            pts[kt] = pt
                        if step >= SKEW:
                            kt2 = step - SKEW
                            off2 = max(0, (kt2 - 4 * qc) * P)
                            pt2 = pts.pop(kt2)
                            for h, half in ((h_e, 0), (h_o, 64)):
                                nc.tensor.matmul(
                                    acc[h][:, off2:],
                                    vx[kt2][:, h * 65:(h + 1) * 65],
                                    pt2[:, half * 8 + off2:
                                        half * 8 + QCS],
                                    start=(kt2 == 0),
                                    stop=(kt2 == nkt - 1),
                                    skip_group_check=True)
                        if deferred:
                            deferred.pop(0)()
                        budget += rate
                        # keep the PE FIFO clear of fillers around block
                        # boundaries so the score->exp handoff of the next
                        # head pair is never queued behind them
                        quiet = step < 2 or step >= nkt - 1
                        while (not quiet and fi < len(fillers)
                               and budget >= 1.0):
                            fillers[fi]()
                            fi += 1
                            budget -= 1.0
                    # block boundary: copy acc to SBUF right away (frees the
                    # PSUM acc pool for the next pair's AV); defer the rest
                    for u in deferred:
                        u()
                    deferred = []
                    for h, half in ((h_e, 0), (h_o, 64)):
                        accS = sb.tile([65, QCS], f32, name=f"accS{qc}_{h}",
                                       tag="accS", bufs=4)
                        nc.vector.tensor_copy(out=accS, in_=acc[h])
                        deferred.append(mk_norm(hp, h, half, accS))
                    deferred.append(mk_ship(hp))
                    if hp == 3:
                        for u in deferred:
                            u()
                        deferred = []
                while fi < len(fillers):
                    fillers[fi]()
                    fi += 1

            # PE warmup: ~10us of dummy matmuls so the HAM clock gate is
            # released before the first real GEMM phase. memset on vector so
            # the warmup isn't queued behind the weight-preload DMA triggers.
            wrm = sb.tile([P, QCS], bf16, name="wrm", tag="wrm", bufs=1)
            nc.vector.memset(wrm, 0.0)
            for w in range(24):
                wps = pp.tile([P, QCS], f32, name=f"wps{w}", tag="mm1",
                              bufs=2)
                nc.tensor.matmul(wps[:, :], wrm[:, 0:128], wrm[:, :],
                                 start=True, stop=True)

            # weight + x preloads overlap the warmup
            preload_weights()
            prefetch_x()

            # qkv(0) standalone, then attention(qc) interleaved with
            # qkv(qc+1) and cproj(qc-1)
            for u in qkv_units(0):
                u()
            # qkv first in each filler list: its x loads must not queue
            # behind cproj's readbacks (which wait on AllGathers), and
            # cproj's matmuls must not hit the PE FIFO before those
            # AllGathers land. qkv(3)'s v + k ct1-3 defer into attention(3)
            # (not needed there before key-tile 12) to feed its PE.
            emit_attention(0, list(qkv_units(1)))
            emit_attention(1, list(qkv_units(2)) + list(cproj_units(0)))
            emit_attention(2, list(qkv_units(3, "early"))
                           + list(cproj_units(1)))
            emit_attention(3, list(qkv_units(3, "late"))
                           + list(cproj_units(2)), rate=2.5)
            for u in cproj_units(QCN - 1):
                u()
    nc.compile()
    return nc


def _get_nc():
    if "nc" not in _CACHE:
        _CACHE["nc"] = _build()
    return _CACHE["nc"]


def _in_maps(x, c_attn_w, c_proj_w):
    import ml_dtypes
    bf = ml_dtypes.bfloat16
    maps = []
    for c in range(NCORES):
        b, g = c // 2, c % 2
        h0 = g * HPC
        cols = slice(h0 * HD, h0 * HD + ACH)
        maps.append({
            "xt": np.ascontiguousarray(x[b].T).astype(bf),
            "wq": np.ascontiguousarray(c_attn_w[:, :D][:, cols]).astype(bf),
            "wk": np.ascontiguousarray(
                c_attn_w[:, D:2 * D][:, cols]).astype(bf),
            "wv": np.ascontiguousarray(
                c_attn_w[:, 2 * D:][:, cols]).astype(bf),
            "wp": np.ascontiguousarray(
                c_proj_w[:, g * OCW:(g + 1) * OCW]).astype(bf),
        })
    return maps


def _run(inputs, trace=False):
    from concourse.bass_utils import run_bass_kernel_spmd
    x = np.asarray(inputs["x"], np.float32)
    c_attn_w = np.asarray(inputs["c_attn_w"], np.float32)
    c_attn_b = np.asarray(inputs["c_attn_b"], np.float32)
    c_proj_w = np.asarray(inputs["c_proj_w"], np.float32)
    c_proj_b = np.asarray(inputs["c_proj_b"], np.float32)
    assert not np.any(c_attn_b), "nonzero c_attn_b not supported"

    nc = _get_nc()
    res = run_bass_kernel_spmd(nc, _in_maps(x, c_attn_w, c_proj_w),
                               core_ids=list(range(NCORES)), trace=trace)
    out = np.empty((B, S, D), np.float32)
    for c in range(NCORES):
        b, g = c // 2, c % 2
        o = res.results[c]["outp"]
        out[b, :, g * OCW:(g + 1) * OCW] = np.asarray(o, np.float32)
    if np.any(c_proj_b):
        out += c_proj_b
    return out, res


def kernel(**inputs):
    out, _ = _run(inputs, trace=False)
    return out


# revision 47
# speedup vs baseline: 1.0338x; 1.0338x over previous
"""TRN2 Bass kernel for GPT-2 style causal self-attention (B=4, S=2048, D=1024, H=16).

Sharding: 8 cores = 4 batches x 2 head-groups (8 heads each).
Each core computes qkv projections for its (batch, head-group), runs causal
attention for its 8 heads, then the two head-groups' attention outputs are
exchanged pairwise (AllGather per head-pair, spread through the kernel) so each
core computes the c_proj over the FULL 1024 a-channels for its half of the
OUTPUT COLUMNS (wp is pre-sliced per core host-side) -- no collective remains
in the kernel tail.

All matmul operands are bf16 (fp32 PSUM accumulation): same PE rate as f32r but
fast-weight-load kicks in and DMA halves. Weights are preloaded to SBUF once.
Softmax needs no max-subtraction (scores bounded ~|2.7|); masked entries are
zeroed after exp via affine_select restricted to the 128-col diagonal band; the
softmax denominator rides along as a 65th ones-column of V in the AV matmul.
AV matmuls skip fully-masked query columns of diagonal key tiles.
Attention is software-pipelined (scores 2 tiles ahead of AV) and score matmuls
for a head pair run concurrently on disjoint PE row groups via tile_position.
"""
import sys
sys.path.insert(0, "/opt/trn_rl_repo")
import numpy as np

B, S, D, H, HD = 4, 2048, 1024, 16, 64
NCORES = 8
HPC = H // 2          # 8 heads per core
ACH = HPC * HD        # 512 local a-channels
P = 128
QCN = 4               # token chunks
QCS = S // QCN        # 512
FKT = D // P          # 8 feature k-tiles
VW = HPC * (HD + 1)   # 520: per-head 64 v-dims + ones column
SKEW = 2              # attention pipeline skew (score tiles ahead of AV)
OCW = D // 2          # 512 output columns per core

_CACHE = {}


def _build():
    from concourse import bacc, tile, mybir
    f32 = mybir.dt.float32
    bf16 = mybir.dt.bfloat16
    Exp = mybir.ActivationFunctionType.Exp

    nc = bacc.Bacc("TRN2", target_bir_lowering=False, debug=False,
                   num_devices=NCORES)
    xt_e = nc.dram_tensor("xt", [D, S], bf16, kind="ExternalInput")
    wq_e = nc.dram_tensor("wq", [D, ACH], bf16, kind="ExternalInput")
    wk_e = nc.dram_tensor("wk", [D, ACH], bf16, kind="ExternalInput")
    wv_e = nc.dram_tensor("wv", [D, ACH], bf16, kind="ExternalInput")
    wp_e = nc.dram_tensor("wp", [D, OCW], bf16, kind="ExternalInput")
    out_e = nc.dram_tensor("outp", [S, OCW], bf16, kind="ExternalOutput")
    rg = [[0, 1], [2, 3], [4, 5], [6, 7]]

    with tile.TileContext(nc) as tc:
        with tc.tile_pool(name="sb", bufs=1) as sb, \
             tc.tile_pool(name="pp", bufs=1, space="PSUM") as pp, \
             tc.tile_pool(name="dr", bufs=1, space="DRAM") as dr:

            kT = [sb.tile([P, S], bf16, name=f"kTr{i}", tag="kT", bufs=4)
                  for i in range(4)]
            vx = [sb.tile([P, VW], bf16, name=f"vxr{i}", tag="vx", bufs=16)
                  for i in range(16)]
            wq_t = [sb.tile([P, ACH], bf16, name=f"wqr{i}", tag="wq", bufs=8)
                    for i in range(FKT)]
            wk_t = [sb.tile([P, ACH], bf16, name=f"wkr{i}", tag="wk", bufs=8)
                    for i in range(FKT)]
            wv_t = [sb.tile([P, ACH], bf16, name=f"wvr{i}", tag="wv", bufs=8)
                    for i in range(FKT)]
            wp_t = [sb.tile([P, OCW], bf16, name=f"wpr{i}", tag="wp", bufs=8)
                    for i in range(FKT)]

            qt_all = {}    # (qc, ct) -> tile
            aods = {}      # (qc, hp) -> dram tile [2P, TPC]

            def preload_weights():
                for k in range(FKT):
                    nc.scalar.dma_start(out=wq_t[k],
                                        in_=wq_e.ap()[k * P:(k + 1) * P, :])
                for k in range(FKT):
                    nc.gpsimd.dma_start(out=wk_t[k],
                                        in_=wk_e.ap()[k * P:(k + 1) * P, :])
                for k in range(FKT):
                    nc.gpsimd.dma_start(out=wv_t[k],
                                        in_=wv_e.ap()[k * P:(k + 1) * P, :])
                for k in range(FKT):
                    nc.gpsimd.dma_start(out=wp_t[k],
                                        in_=wp_e.ap()[k * P:(k + 1) * P, :])

            _xc = {}

            def qk_group(qc, proj, ct):
                w_t = wq_t if proj == "q" else wk_t
                xc = _xc[qc]
                mm_ps = pp.tile([P, QCS], f32, name=f"{proj}ps{qc}_{ct}",
                                tag="mm1", bufs=2)
                for k in range(FKT):
                    def mm(k=k, mm_ps=mm_ps, w_t=w_t, ct=ct, xck=xc[k]):
                        nc.tensor.matmul(
                            mm_ps[:, :], w_t[k][:, ct * P:(ct + 1) * P],
                            xck[:, :], start=(k == 0), stop=(k == FKT - 1))
                    yield mm
                if proj == "q":
                    qt = sb.tile([P, QCS], bf16, name=f"qt{qc}_{ct}",
                                 tag="qt", bufs=8)
                    qt_all[qc, ct] = qt

                    def cp(qt=qt, mm_ps=mm_ps):
                        nc.vector.tensor_copy(out=qt, in_=mm_ps)
                    yield cp
                else:
                    def cp(ct=ct, mm_ps=mm_ps, qc=qc):
                        nc.vector.tensor_copy(
                            out=kT[ct][:, qc * QCS:(qc + 1) * QCS],
                            in_=mm_ps)
                    yield cp

            def v_group(qc, vt):
                xc = _xc[qc]
                v_ps = pp.tile([P, ACH], f32, name=f"vps{qc}_{vt}",
                               tag="mm1", bufs=2)
                for k in range(FKT):
                    def mm(k=k, v_ps=v_ps, xck=xc[k], vt=vt):
                        nc.tensor.matmul(v_ps[:, :],
                                         xck[:, vt * P:(vt + 1) * P],
                                         wv_t[k][:, :], start=(k == 0),
                                         stop=(k == FKT - 1))
                    yield mm

                def vcp(qc=qc, vt=vt, v_ps=v_ps):
                    vxt = vx[qc * 4 + vt]
                    v3 = vxt.rearrange("p (h w) -> p h w", w=HD + 1)
                    nc.gpsimd.memset(v3[:, :, HD:HD + 1], 1.0)
                    nc.vector.tensor_copy(
                        out=v3[:, :, 0:HD],
                        in_=v_ps.rearrange("p (h d) -> p h d", d=HD))
                yield vcp

            def prefetch_x():
                """All of x is only 4MB bf16: load every chunk up front so
                the sync queue carries nothing but cproj readbacks later.
                x(2)/x(3) go on the gpsimd queue BEHIND the weight preloads:
                their transfers then serialize after the weights, keeping the
                startup DMA burst off the critical wq/x(0) path."""
                for qc in range(QCN):
                    _xc[qc] = [sb.tile([P, QCS], bf16, name=f"xc{qc}_{k}",
                                       tag="xc", bufs=32)
                               for k in range(FKT)]
                for qc in range(QCN):
                    eng = nc.sync if qc < 2 else nc.gpsimd
                    for k in range(FKT):
                        eng.dma_start(
                            out=_xc[qc][k],
                            in_=xt_e.ap()[k * P:(k + 1) * P,
                                          qc * QCS:(qc + 1) * QCS])

            def qkv_units(qc, part="all"):
                """Emission closures for the qkv phase of qc. part='early'
                emits q projections + k ct0; part='late' the rest (v first,
                then k ct1-3) -- deferred into attention(qc) where they are
                not needed before key-tile 4qc."""
                if part == "all":
                    for ct in range(4):
                        yield from qk_group(qc, "q", ct)
                    for ct in range(4):
                        yield from qk_group(qc, "k", ct)
                    for vt in range(4):
                        yield from v_group(qc, vt)
                elif part == "early":
                    for ct in range(4):
                        yield from qk_group(qc, "q", ct)
                    yield from qk_group(qc, "k", 0)
                else:
                    for vt in range(4):
                        yield from v_group(qc, vt)
                    for ct in range(1, 4):
                        yield from qk_group(qc, "k", ct)

            def cproj_units(qc):
                """Generator of closures for c_proj of qc: read back the
                AllGather'd attention outputs (both head-groups, all 512
                tokens) and compute c_proj for this core's OCW out-columns."""
                ao = {}
                for j in range(2):
                    for hp in range(4):
                        ao[j, hp] = sb.tile([P, QCS], bf16,
                                            name=f"ao{qc}_{j}_{hp}",
                                            tag="ao", bufs=16)

                def rb(hp):
                    for j in range(2):
                        nc.sync.dma_start(
                            out=ao[j, hp],
                            in_=aods[qc, hp][j * P:(j + 1) * P, :])
                for hp in range(4):
                    yield lambda hp=hp: rb(hp)

                def st_(qc, tt, po):
                    pout = sb.tile([P, OCW], bf16, name=f"pout{qc}_{tt}",
                                   tag="pout", bufs=4)
                    nc.vector.tensor_copy(out=pout, in_=po)
                    nc.gpsimd.dma_start(
                        out=out_e.ap()[qc * QCS + tt * P:
                                       qc * QCS + (tt + 1) * P, :],
                        in_=pout)

                if qc < QCN - 1:
                    # tt-major: AllGathers finished long ago, 8-matmul
                    # bursts per PSUM tile keep the mm1 pool fluid
                    for tt in range(4):
                        po = pp.tile([P, OCW], f32, name=f"po{qc}_{tt}",
                                     tag="mm1", bufs=2)
                        for hp in range(4):
                            for j in range(2):
                                def mm(j=j, hp=hp, po=po, tt=tt):
                                    nc.tensor.matmul(
                                        po[:, :],
                                        ao[j, hp][:, tt * P:(tt + 1) * P],
                                        wp_t[j * 4 + hp][:, :],
                                        start=(hp == 0 and j == 0),
                                        stop=(hp == 3 and j == 1))
                                yield mm
                        yield lambda qc=qc, tt=tt, po=po: st_(qc, tt, po)
                else:
                    # tail chunk: hp-major over tt pairs so only the final
                    # head pair's matmuls wait on the last AllGather
                    for tp in range(2):
                        pos = {tt: pp.tile([P, OCW], f32,
                                           name=f"po{qc}_{tt}", tag="mm1",
                                           bufs=2)
                               for tt in (2 * tp, 2 * tp + 1)}
                        for hp in range(4):
                            if tp == 0 and hp == 3:
                                # warm-keeper: the next matmuls wait on the
                                # last AllGather; keep the HAM clock gate
                                # open with dummy matmuls meanwhile
                                def wk():
                                    for w in range(48):
                                        dps = pp.tile(
                                            [65, QCS], f32,
                                            name=f"dps{qc}_{w}",
                                            tag="acc", bufs=2)
                                        nc.tensor.matmul(
                                            dps[:, :], wrm[:, 0:65],
                                            wrm[:, :], start=True,
                                            stop=True)
                                yield wk
                            for j in range(2):
                                for tt in (2 * tp, 2 * tp + 1):
                                    def mm(j=j, hp=hp, tt=tt, pos=pos):
                                        nc.tensor.matmul(
                                            pos[tt][:, :],
                                            ao[j, hp][:, tt * P:
                                                      (tt + 1) * P],
                                            wp_t[j * 4 + hp][:, :],
                                            start=(hp == 0 and j == 0),
                                            stop=(hp == 3 and j == 1))
                                    yield mm
                        for tt in (2 * tp, 2 * tp + 1):
                            yield (lambda qc=qc, tt=tt, pos=pos:
                                   st_(qc, tt, pos[tt]))

            def emit_attention(qc, fillers, rate=2.2):
                """Emit attention for qc, interleaving filler closures at
                ~rate units per pipeline step. Each head pair's softmax
                normalization + partner-exchange is deferred into the next
                head pair's early steps: only a fast acc->SBUF copy happens
                at the block boundary, so the PSUM acc pool and the gpsimd
                queue never stall the next block's matmuls/selects."""
                nkt = 4 * qc + 4
                fi = 0
                budget = 0.0
                at_tiles = [sb.tile([P, QCS], bf16, name=f"at{qc}_{j}",
                                    tag="at", bufs=6) for j in range(4)]
                deferred = []

                def mk_norm(hp, h, half, accS_h):
                    def rb_():
                        rsum = sb.tile([1, QCS], f32, name=f"rsum{qc}_{h}",
                                       tag="rs", bufs=4)
                        rs_t = sb.tile([1, QCS], f32, name=f"rst{qc}_{h}",
                                       tag="rs2", bufs=4)
                        rb_t = sb.tile([64, QCS], f32, name=f"rb{qc}_{h}",
                                       tag="rb", bufs=4)
                        # recip is a custom DVE op: give it a partition-0
                        # based operand, not a base_partition=64 slice
                        nc.vector.tensor_copy(out=rsum,
                                              in_=accS_h[64:65, :])
                        nc.vector.reciprocal_approx_fast(
                            out=rs_t, in_=rsum)
                        nc.gpsimd.partition_broadcast(rb_t[:, :],
                                                      rs_t[:, :])
                        nc.vector.tensor_tensor(
                            out=at_tiles[hp][half:half + 64, :],
                            in0=accS_h[0:64, :], in1=rb_t[:, :],
                            op=mybir.AluOpType.mult)
                    return rb_

                def mk_ship(hp):
                    def ship():
                        # AllGather -> aod rows [0:P] = member 0's
                        # (head-group 0), rows [P:2P] = member 1's.
                        atd = dr.tile([P, QCS], bf16, name=f"atd{qc}_{hp}",
                                      tag=f"atd{qc}_{hp}")
                        aod = dr.tile([2 * P, QCS], bf16,
                                      name=f"aod{qc}_{hp}",
                                      tag=f"aod{qc}_{hp}")
                        nc.gpsimd.dma_start(out=atd[:, :], in_=at_tiles[hp])
                        nc.gpsimd.collective_compute(
                            "AllGather", mybir.AluOpType.bypass,
                            ins=[atd.opt()], outs=[aod.opt()],
                            replica_groups=rg)
                        aods[qc, hp] = aod
                    return ship

                for hp in range(4):
                    h_e, h_o = 2 * hp, 2 * hp + 1
                    acc = {}
                    for h, half in ((h_e, 0), (h_o, 64)):
                        acc[h] = pp.tile([65, QCS], f32, name=f"acc{qc}_{h}",
                                         tag="acc", bufs=2)
                    pts = {}
                    for step in range(nkt + SKEW):
                        if step < nkt:
                            kt = step
                            # both heads' score tiles share one 2-bank PSUM
                            # tile; a single exp covers the pair
                            st = pp.tile([P, 2 * QCS], f32,
                                         name=f"st{qc}_{hp}_{kt}",
                                         tag="st", bufs=2)
                            for h, half in ((h_e, 0), (h_o, 64)):
                                nc.tensor.matmul(
                                    st[:, half * 8:half * 8 + QCS],
                                    kT[hp][half:half + 64,
                                           kt * P:(kt + 1) * P],
                                    qt_all[qc, hp][half:half + 64, :],
                                    start=True, stop=True,
                                    tile_position=(half, 0))
                            pt = sb.tile([P, 2 * QCS], bf16,
                                         name=f"pt{qc}_{hp}_{kt}",
                                         tag="pt", bufs=4)
                            nc.scalar.activation(out=pt, in_=st,
                                                 func=Exp, scale=0.125)
                            if kt >= 4 * qc:
                                off = (kt - 4 * qc) * P
                                for half in (0, 64):
                                    nc.gpsimd.affine_select(
                                        out=pt[:, half * 8 + off:
                                               half * 8 + off + P],
                                        in_=pt[:, half * 8 + off:
                                               half * 8 + off + P],
                                        compare_op=mybir.AluOpType.is_ge,
                                        fill=0.0, base=0,
                                        pattern=[[1, P]],
                                        channel_multiplier=-1)
                            pts[kt] = pt
                        if step >= SKEW:
                            kt2 = step - SKEW
                            off2 = max(0, (kt2 - 4 * qc) * P)
                            pt2 = pts.pop(kt2)
                            for h, half in ((h_e, 0), (h_o, 64)):
                                nc.tensor.matmul(
                                    acc[h][:, off2:],
                                    vx[kt2][:, h * 65:(h + 1) * 65],
                                    pt2[:, half * 8 + off2:
                                        half * 8 + QCS],
                                    start=(kt2 == 0),
                                    stop=(kt2 == nkt - 1),
                                    skip_group_check=True)
                        if deferred:
                            deferred.pop(0)()
                        budget += rate
                        # keep the PE FIFO clear of fillers around block
                        # boundaries so the score->exp handoff of the next
                        # head pair is never queued behind them
                        quiet = step < 2 or step >= nkt - 1
                        while (not quiet and fi < len(fillers)
                               and budget >= 1.0):
                            fillers[fi]()
                            fi += 1
                            budget -= 1.0
                    # block boundary: copy acc to SBUF right away (frees the
                    # PSUM acc pool for the next pair's AV); defer the rest
                    for u in deferred:
                        u()
                    deferred = []
                    for h, half in ((h_e, 0), (h_o, 64)):
                        accS = sb.tile([65, QCS], f32, name=f"accS{qc}_{h}",
                                       tag="accS", bufs=4)
                        nc.vector.tensor_copy(out=accS, in_=acc[h])
                        deferred.append(mk_norm(hp, h, half, accS))
                    deferred.append(mk_ship(hp))
                    if hp == 3:
                        for u in deferred:
                            u()
                        deferred = []
                while fi < len(fillers):
                    fillers[fi]()
                    fi += 1

            # PE warmup: ~10us of dummy matmuls so the HAM clock gate is
            # released before the first real GEMM phase. memset on vector so
            # the warmup isn't queued behind the weight-preload DMA triggers.
            wrm = sb.tile([P, QCS], bf16, name="wrm", tag="wrm", bufs=1)
            nc.vector.memset(wrm, 0.0)
            for w in range(24):
                wps = pp.tile([P, QCS], f32, name=f"wps{w}", tag="mm1",
                              bufs=2)
                nc.tensor.matmul(wps[:, :], wrm[:, 0:128], wrm[:, :],
                                 start=True, stop=True)

            # weight + x preloads overlap the warmup
            preload_weights()
            prefetch_x()

            # qkv(0) standalone, then attention(qc) interleaved with
            # qkv(qc+1) and cproj(qc-1)
            for u in qkv_units(0):
                u()
            # qkv first in each filler list: its x loads must not queue
            # behind cproj's readbacks (which wait on AllGathers), and
            # cproj's matmuls must not hit the PE FIFO before those
            # AllGathers land. qkv(3)'s v + k ct1-3 defer into attention(3)
            # (not needed there before key-tile 12) to feed its PE.
            emit_attention(0, list(qkv_units(1)))
            emit_attention(1, list(qkv_units(2)) + list(cproj_units(0)))
            emit_attention(2, list(qkv_units(3, "early"))
                           + list(cproj_units(1)))
            emit_attention(3, list(qkv_units(3, "late"))
                           + list(cproj_units(2)), rate=2.5)
            for u in cproj_units(QCN - 1):
                u()
    nc.compile()
    return nc


def _get_nc():
    if "nc" not in _CACHE:
        _CACHE["nc"] = _build()
    return _CACHE["nc"]


def _in_maps(x, c_attn_w, c_proj_w):
    import ml_dtypes
    bf = ml_dtypes.bfloat16
    maps = []
    for c in range(NCORES):
        b, g = c // 2, c % 2
        h0 = g * HPC
        cols = slice(h0 * HD, h0 * HD + ACH)
        maps.append({
            "xt": np.ascontiguousarray(x[b].T).astype(bf),
            "wq": np.ascontiguousarray(c_attn_w[:, :D][:, cols]).astype(bf),
            "wk": np.ascontiguousarray(
                c_attn_w[:, D:2 * D][:, cols]).astype(bf),
            "wv": np.ascontiguousarray(
                c_attn_w[:, 2 * D:][:, cols]).astype(bf),
            "wp": np.ascontiguousarray(
                c_proj_w[:, g * OCW:(g + 1) * OCW]).astype(bf),
        })
    return maps


def _run(inputs, trace=False):
    from concourse.bass_utils import run_bass_kernel_spmd
    x = np.asarray(inputs["x"], np.float32)
    c_attn_w = np.asarray(inputs["c_attn_w"], np.float32)
    c_attn_b = np.asarray(inputs["c_attn_b"], np.float32)
    c_proj_w = np.asarray(inputs["c_proj_w"], np.float32)
    c_proj_b = np.asarray(inputs["c_proj_b"], np.float32)
    assert not np.any(c_attn_b), "nonzero c_attn_b not supported"

    nc = _get_nc()
    res = run_bass_kernel_spmd(nc, _in_maps(x, c_attn_w, c_proj_w),
                               core_ids=list(range(NCORES)), trace=trace)
    out = np.empty((B, S, D), np.float32)
    for c in range(NCORES):
        b, g = c // 2, c % 2
        o = res.results[c]["outp"]
        out[b, :, g * OCW:(g + 1) * OCW] = np.asarray(o, np.float32)
    if np.any(c_proj_b):
        out += c_proj_b
    return out, res


def kernel(**inputs):
    out, _ = _run(inputs, trace=False)
    return out
